# revision 72
# baseline (speedup 1.0000x reference)
"""Biaffine attention kernel for Trainium2, data-parallel over 8 NeuronCores.

Math (per batch b, X = x[b] (128, L), Y = y[b]):
    xp = Wf X + bf 1^T,  yp = Wa Y + ba 1^T
    scores = xp yp^T
           = Wf (X Y^T) Wa^T + (Wf X 1) ba^T + bf (Wa Y 1)^T + L bf ba^T
    attn = softmax(scores, -1) / sqrt(L)
    out  = attn (xp + yp) = (attn Wf) X + (attn Wa) Y + (attn (bf+ba)) 1^T

Distribution: batch dim (32) sharded 4-per-core across 8 cores; weights
replicated. No collectives.

Implementation notes (per core):
  - x/y are cast to fp16 on the host; all HBM traffic is fp16 (in 4 MiB +
    out 2 MiB per batch vs 12 MiB for fp32), out is upcast host-side.
  - "gen": xpT chunks = matmul(lhsT=X_c, rhs=WfT) produce the transposed
    projections directly from the natural-layout input — this fuses the
    projection and the transpose into one PE pass (128 N=128 matmuls per
    batch instead of 64 proj-chunk + 128 transpose ops). PSUM evacuation
    is split between ScalarE (x) and VectorE (y).
  - scores accumulate over the 64 transposed chunks into one PSUM bank;
    the three bias rank-1 terms are added exactly via two k=1 matmuls
    (u ba^T and bf (Wa sy + L ba)^T) from host-precomputed rows.
  - out = AfT.T @ X + AaT.T @ Y consumed from the still-resident raw fp16
    inputs (AfT = (attn Wf)^T etc., two small matmuls per batch), with the
    attn(bf+ba) column folded in as the ScalarE PSUM-evacuation bias.
  - software pipelining: gen(b+1) issues between softmax(b) and the PE
    tail of batch b; the second-to-last tail is deferred past the last
    softmax; small constants ride the SWDGE ring so nothing queues behind
    the input stream.
"""

import numpy as np

P = 128
L = 8192
B = 32
NCORES = 8
BPC = B // NCORES  # batches per core
SQRT_L = float(np.sqrt(float(L)))

GEN_GRP = 4  # gen chunks per PSUM evacuation (4*128 fp32 = 1 bank)
OUT_TILE = 2048  # out staging tile (fp16) per DMA store
OUT_CHUNK = 512  # out matmul free dim / PSUM bank
IN_PIECE = 2048  # input DMA piece (cols)


def _patch_tail_drain(tile, mybir, ScopedClock):
    """This container's walrus rejects >1 sync wait on the kernel-tail Drain
    (setupSyncWait: 'Too many sync wait commands'). Spread the tail-drain
    waits across a chain of drains, one wait each."""
    if getattr(tile.TileContext, "_drain_split_patched", False):
        return

    def _split_drain_and_barrier(self, tick_clock, wait_clock):
        nc = self.nc
        drain_inst = nc.sync.drain()
        wait_clock.add_sem_waits(
            drain_inst.ins, ScopedClock({None: tick_clock.global_clock})
        )
        si = drain_inst.ins.sync_info
        if si is not None and si.on_wait is not None and len(si.on_wait) > 1:
            waits = list(si.on_wait)
            si.on_wait = waits[:1]
            # spread the remaining waits across all engines as parallel
            # single-wait NOP chains (walrus allows only one wait per
            # instruction; a serial chain on SP costs ~4us of kernel tail)
            engines = [nc.tensor, nc.vector, nc.scalar, nc.gpsimd, nc.sync]
            for i, w in enumerate(waits[1:]):
                extra = engines[i % len(engines)].nop(nofuse=True)
                esi = extra.ins.sync_info
                if esi is None:
                    extra.ins.sync_info = mybir.SyncInfo(on_wait=[w], on_update=[])
                else:
                    ow = list(esi.on_wait) if esi.on_wait else []
                    ow.append(w)
                    esi.on_wait = ow
        nc.all_engine_barrier()
        assert self.sems is not None
        popped = nc._tile_sem_poison_stack.pop()
        assert popped is self._sem_poison
        nc.clear_and_free_semaphores(list(self.sems.allocated().values()))
        nc.all_engine_barrier()

    tile.TileContext._drain_and_barrier = _split_drain_and_barrier
    tile.TileContext._drain_split_patched = True


def _split_excess_waits(nc, mybir, max_waits=1):
    """Walrus in this container rejects instructions carrying more than a
    couple of sync waits ('Too many sync wait commands'). Hoist excess waits
    onto dedicated same-engine NoOps inserted just before the instruction."""
    ctr = 0
    for blk in nc.m.functions[0].blocks:
        new_insts = []
        for inst in blk.instructions:
            si = inst.sync_info
            if si is not None and si.on_wait and len(si.on_wait) > max_waits:
                waits = list(si.on_wait)
                excess, keep = waits[:-max_waits], waits[-max_waits:]
                si.on_wait = keep
                for i in range(0, len(excess), max_waits):
                    ctr += 1
                    nop = mybir.InstNoOp(
                        name=f"I-waitsplit-{ctr}",
                        sync_info=mybir.SyncInfo(
                            on_wait=excess[i : i + max_waits], on_update=[]
                        ),
                        bass_nofuse=True,
                        engine=inst.engine,
                    )
                    nc.register_instruction(nop)
                    new_insts.append(nop)
            new_insts.append(inst)
        blk.instructions = new_insts


def build_nc(bpc=BPC, seq=L):
    import concourse.bass as bass
    import concourse.mybir as mybir
    import concourse.tile as tile
    from concourse.masks import make_identity
    from concourse.vector_clock import ScopedClock

    _patch_tail_drain(tile, mybir, ScopedClock)

    f32 = mybir.dt.float32
    f16 = mybir.dt.float16
    AF = mybir.ActivationFunctionType
    ALU = mybir.AluOpType
    AX = mybir.AxisListType

    sqrt_l = float(np.sqrt(float(seq)))
    ntr = seq // P  # number of 128-col chunks
    ngrp = ntr // GEN_GRP
    nout = seq // OUT_TILE
    cpo = OUT_TILE // OUT_CHUNK
    npc = seq // IN_PIECE  # input dma pieces

    nc = bass.Bass("TRN2", target_bir_lowering=False, debug=False)
    x_d = nc.dram_tensor("x", [bpc, P, seq], f16, kind="ExternalInput").ap()
    y_d = nc.dram_tensor("y", [bpc, P, seq], f16, kind="ExternalInput").ap()
    wf_d = nc.dram_tensor("wf", [P, P], f32, kind="ExternalInput").ap()
    bf_d = nc.dram_tensor("bf", [P], f32, kind="ExternalInput").ap()
    wa_d = nc.dram_tensor("wa", [P, P], f32, kind="ExternalInput").ap()
    ba_d = nc.dram_tensor("ba", [P], f32, kind="ExternalInput").ap()
    # host-precomputed bias rank-1 rows: u = Wf (X 1), w = Wa (Y 1) + L ba
    urow_d = nc.dram_tensor("urow", [1, bpc * P], f16, kind="ExternalInput").ap()
    wrow_d = nc.dram_tensor("wrow", [1, bpc * P], f16, kind="ExternalInput").ap()
    out_d = nc.dram_tensor("out", [bpc, P, seq], f16, kind="ExternalOutput").ap()

    def load_inputs(b, x_t, y_t):
        if b == 0:
            # graduated pieces: compute can start after ~256 KiB
            edges = sorted({min(e, seq) for e in (0, 512, 1024, 2048, 4096, seq)})
        else:
            edges = [p_ * IN_PIECE for p_ in range(npc + 1)]
        for e0, e1 in zip(edges[:-1], edges[1:]):
            ps_ = slice(e0, e1)
            nc.sync.dma_start(x_t[:, ps_], x_d[b, :, ps_])
            if b == 0:
                # parallel ring for the critical first batch
                nc.scalar.dma_start(y_t[:, ps_], y_d[b, :, ps_])
            else:
                nc.sync.dma_start(y_t[:, ps_], y_d[b, :, ps_])

    with tile.TileContext(nc) as tc:
        with (
            tc.tile_pool(name="consts", bufs=1) as consts,
            tc.tile_pool(name="xin", bufs=3) as xin_pool,
            tc.tile_pool(name="acts", bufs=2) as acts_pool,
            tc.tile_pool(name="sm", bufs=2) as sm_pool,
            tc.tile_pool(name="outs", bufs=3) as out_pool,
            tc.tile_pool(name="pgen", bufs=3, space="PSUM") as psum_gen,
            tc.tile_pool(name="psc", bufs=2, space="PSUM") as psum_sc,
            tc.tile_pool(name="pout", bufs=3, space="PSUM") as psum_out,
        ):
            # ---- critical constants first (ACT ring), then the batch-0
            # preload pieces, then the non-critical constants ----
            wf_nat = consts.tile([P, P], f32)
            nc.scalar.dma_start(wf_nat, wf_d)
            wa_nat = consts.tile([P, P], f32)
            nc.scalar.dma_start(wa_nat, wa_d)
            id32 = consts.tile([P, P], f32)
            make_identity(nc, id32)
            ids16 = consts.tile([P, P], f16)
            make_identity(nc, ids16)

            xy_tiles = {}

            def issue_load(b):
                if b >= bpc:
                    return
                x_t = xin_pool.tile([P, seq], f16, tag="x_t", name="x_t")
                y_t = xin_pool.tile([P, seq], f16, tag="y_t", name="y_t")
                load_inputs(b, x_t, y_t)
                xy_tiles[b] = (x_t, y_t)

            # small constants on the (otherwise idle until the first store)
            # SWDGE ring so they land immediately and never queue behind the
            # batch-0 input pieces
            bias_f = consts.tile([P, 1], f32)
            nc.gpsimd.dma_start(bias_f, bf_d.rearrange("(p o) -> p o", o=1))
            bias_a = consts.tile([P, 1], f32)
            nc.gpsimd.dma_start(bias_a, ba_d.rearrange("(p o) -> p o", o=1))
            bf_row = consts.tile([1, P], f32)
            nc.gpsimd.dma_start(bf_row, bf_d.rearrange("(o p) -> o p", o=1))
            ba_row = consts.tile([1, P], f32)
            nc.gpsimd.dma_start(ba_row, ba_d.rearrange("(o p) -> o p", o=1))
            urows_t = consts.tile([1, bpc * P], f16)
            nc.gpsimd.dma_start(urows_t, urow_d)
            wrows_t = consts.tile([1, bpc * P], f16)
            nc.gpsimd.dma_start(wrows_t, wrow_d)

            issue_load(0)

            for b0 in range(1, min(3, bpc)):
                issue_load(b0)

            # fp16 natural weights (lhsT for the AfT/AaT matmuls)
            wf16 = consts.tile([P, P], f16)
            nc.vector.tensor_copy(out=wf16, in_=wf_nat)
            wa16 = consts.tile([P, P], f16)
            nc.vector.tensor_copy(out=wa16, in_=wa_nat)
            # fp16 transposed weights ([in, out], rhs of the gen matmuls)
            wfT16 = consts.tile([P, P], f16)
            waT16 = consts.tile([P, P], f16)
            for nat, tsp in ((wf_nat, wfT16), (wa_nat, waT16)):
                pwb = psum_sc.tile([P, 4 * P], f32, tag="ps", name="pwb")
                pw = pwb[:, 0:P]
                nc.tensor.transpose(pw, nat, id32)
                nc.vector.tensor_copy(out=tsp, in_=pw)
            # fp16 bias rows, c = bf+ba column (fp16 rhs; fp32 for reference)
            bf_row16 = consts.tile([1, P], f16)
            nc.vector.tensor_copy(out=bf_row16, in_=bf_row)
            ba_row16 = consts.tile([1, P], f16)
            nc.vector.tensor_copy(out=ba_row16, in_=ba_row)
            c_col16 = consts.tile([P, 1], f16)
            nc.vector.scalar_tensor_tensor(
                out=c_col16, in0=bias_f, scalar=1.0, in1=bias_a,
                op0=ALU.mult, op1=ALU.add,
            )

            def do_gen_scores(b):
                # transposed projections (no bias):
                # xpT[:, c*128:(c+1)*128] = X_c^T @ Wf^T   (l on partitions),
                # with the scores accumulation interleaved one group behind —
                # the gen phase is evacuation-bound, so the scores matmuls
                # fill the PE stalls at the gen-group boundaries.
                x_t, y_t = xy_tiles[b]
                xpT = acts_pool.tile([P, seq], f16, tag="xpT", name="xpT")
                ypT = acts_pool.tile([P, seq], f16, tag="ypT", name="ypT")
                # One PSUM bank per batch: cols 0-127 scores, the rest is
                # sliced by do_tail for the small post-softmax matmuls.
                pbank = psum_sc.tile([P, 4 * P], f32, tag="ps", name="pbank")
                ps = pbank[:, 0:P]

                def gen_group(g):
                    for src, wT, dst, eng in (
                        (x_t, wfT16, xpT, nc.scalar),
                        (y_t, waT16, ypT, nc.vector),
                    ):
                        pg = psum_gen.tile([P, GEN_GRP * P], f32, tag="pg", name="pg")
                        for t in range(GEN_GRP):
                            c = g * GEN_GRP + t
                            cs = slice(c * P, (c + 1) * P)
                            ts_ = slice(t * P, (t + 1) * P)
                            nc.tensor.matmul(
                                pg[:, ts_], src[:, cs], wT,
                                start=True, stop=True,
                            )
                        gs = slice(g * GEN_GRP * P, (g + 1) * GEN_GRP * P)
                        if eng is nc.scalar:
                            nc.scalar.activation(
                                out=dst[:, gs], in_=pg, func=AF.Identity
                            )
                        else:
                            nc.vector.tensor_copy(out=dst[:, gs], in_=pg)

                def score_group(g):
                    for t in range(GEN_GRP):
                        c = g * GEN_GRP + t
                        cs = slice(c * P, (c + 1) * P)
                        nc.tensor.matmul(
                            ps, xpT[:, cs], ypT[:, cs],
                            start=(c == 0), stop=False,
                            skip_group_check=True,
                        )

                for g in range(ngrp):
                    gen_group(g)
                    if g >= 1:
                        score_group(g - 1)
                score_group(ngrp - 1)
                bs = slice(b * P, (b + 1) * P)
                nc.tensor.matmul(
                    ps, urows_t[:, bs], ba_row16, start=False, stop=False,
                    skip_group_check=True,
                )
                nc.tensor.matmul(
                    ps, bf_row16, wrows_t[:, bs], start=False, stop=True,
                    skip_group_check=True,
                )
                return pbank

            def do_softmax_a(b, pbank):
                # softmax part 1: max-reduce + exp (DVE + ACT)
                ps = pbank[:, 0:P]
                negmx = sm_pool.tile([P, 1], f32, tag="negmx", name="negmx")
                nc.vector.tensor_reduce(
                    out=negmx, in_=ps, axis=AX.X, op=ALU.max, negate=True
                )
                e = sm_pool.tile([P, P], f32, tag="e", name="e")
                se = sm_pool.tile([P, 1], f32, tag="se", name="se")
                nc.scalar.activation(
                    out=e, in_=ps, func=AF.Exp, bias=negmx, scale=1.0, accum_out=se
                )
                return e, se

            def do_softmax_b(b, e, se):
                # softmax part 2: normalize (DVE only)
                sse = sm_pool.tile([P, 1], f32, tag="sse", name="sse")
                nc.vector.tensor_scalar_mul(sse, se, sqrt_l)
                rcp = sm_pool.tile([P, 1], f32, tag="rcp", name="rcp")
                nc.vector.reciprocal(rcp, sse)
                attn = sm_pool.tile([P, P], f16, tag="attn", name="attn")
                nc.vector.tensor_scalar_mul(attn, e, rcp)
                return attn

            def do_softmax(b, pbank):
                e, se = do_softmax_a(b, pbank)
                return do_softmax_b(b, e, se)

            def do_tail(b, attn, pbank, out_tile=OUT_TILE):
                x_t, y_t = xy_tiles.pop(b)
                nout_ = seq // out_tile
                cpo_ = out_tile // OUT_CHUNK
                pat = pbank[:, P : P + P // 2].bitcast(f16)
                nc.tensor.transpose(pat, attn, ids16)
                attnT = sm_pool.tile([P, P], f16, tag="attnT", name="attnT")
                nc.vector.tensor_copy(out=attnT, in_=pat)

                # ---- phase D: folded output weights ----
                # AfT = Wf^T attn^T = (attn Wf)^T ; attnc = attn (bf+ba)
                paf = pbank[:, 2 * P - P // 2 : 3 * P - P // 2]
                nc.tensor.matmul(paf, wf16, attnT, start=True, stop=True)
                afT16 = sm_pool.tile([P, P], f16, tag="afT", name="afT")
                nc.vector.tensor_copy(out=afT16, in_=paf)
                paa = pbank[:, 3 * P - P // 2 : 4 * P - P // 2]
                nc.tensor.matmul(paa, wa16, attnT, start=True, stop=True)
                aaT16 = sm_pool.tile([P, P], f16, tag="aaT", name="aaT")
                nc.vector.tensor_copy(out=aaT16, in_=paa)
                pac = pbank[:, 4 * P - P // 2 : 4 * P - P // 2 + 1]
                nc.tensor.matmul(pac, attnT, c_col16, start=True, stop=True)
                attnc = sm_pool.tile([P, 1], f32, tag="attnc", name="attnc")
                nc.vector.tensor_copy(out=attnc, in_=pac)

                # ---- phase E: out = AfT.T @ X + AaT.T @ Y + attnc ----
                otag = "ot" if out_tile == OUT_TILE else f"ot{out_tile}"
                for h in range(nout_):
                    ot = out_pool.tile([P, out_tile], f16, tag=otag, name="ot")
                    for cc in range(cpo_):
                        c0 = h * out_tile + cc * OUT_CHUNK
                        cs = slice(c0, c0 + OUT_CHUNK)
                        po = psum_out.tile([P, OUT_CHUNK], f32, tag="po", name="po")
                        nc.tensor.matmul(
                            po, afT16, x_t[:, cs], start=True, stop=False
                        )
                        nc.tensor.matmul(
                            po, aaT16, y_t[:, cs], start=False, stop=True
                        )
                        ots = ot[:, cc * OUT_CHUNK : (cc + 1) * OUT_CHUNK]
                        # split PSUM evacuation between ACT and DVE
                        if cc % 2 == 0:
                            nc.scalar.activation(
                                out=ots, in_=po, func=AF.Identity, bias=attnc
                            )
                        else:
                            nc.vector.tensor_scalar_add(ots, po, attnc)
                    hs = slice(h * out_tile, (h + 1) * out_tile)
                    if b == bpc - 1:
                        # last batch: ACT HWDGE ring (idle by then) — keeps
                        # the slow SWDGE drain off the kernel's critical tail
                        nc.scalar.dma_start(out_d[b, :, hs], ot)
                    else:
                        # stores issue from the gpsimd SWDGE ring: ACT is
                        # busy with evacuations, the SP ring with loads
                        nc.gpsimd.dma_start(out_d[b, :, hs], ot)

            # ---- software-pipelined driver ----
            # gen(b+1) is issued between softmax(b) and the PE tail of b,
            # so the PE stays busy during the serial softmax chain. The
            # second-to-last tail is deferred past the last softmax so that
            # one is hidden too.
            pbank = do_gen_scores(0)
            stash = None
            for b in range(bpc):
                if b + 1 < bpc:
                    attn = do_softmax(b, pbank)
                    issue_load(b + 3)
                    next_pbank = do_gen_scores(b + 1)
                    if b == bpc - 2:
                        stash = (attn, pbank)
                    else:
                        do_tail(b, attn, pbank)
                    pbank = next_pbank
                else:
                    # last batch: exp first, then the deferred tail runs on
                    # the PE while the normalize half completes
                    e, se = do_softmax_a(b, pbank)
                    if stash is not None:
                        do_tail(b - 1, stash[0], stash[1])
                    attn = do_softmax_b(b, e, se)
                    do_tail(b, attn, pbank, out_tile=1024)

    _split_excess_waits(nc, mybir, max_waits=1)
    return nc


_nc_cache = {}


def _get_nc():
    key = (BPC, L)
    if key not in _nc_cache:
        _nc_cache[key] = build_nc(BPC, L)
    return _nc_cache[key]


def make_in_maps(x, y, Wf, bf, Wa, ba):
    x16 = np.asarray(x).astype(np.float16)
    y16 = np.asarray(y).astype(np.float16)
    Wf = np.ascontiguousarray(np.asarray(Wf, dtype=np.float32))
    bf = np.ascontiguousarray(np.asarray(bf, dtype=np.float32))
    Wa = np.ascontiguousarray(np.asarray(Wa, dtype=np.float32))
    ba = np.ascontiguousarray(np.asarray(ba, dtype=np.float32))

    # Bias rank-1 rows for the scores (exact, vs the fp16-quantized inputs):
    # scores = Wf G Wa^T + u ba^T + bf w^T with u = Wf (X 1), w = Wa (Y 1) + L ba
    sx = x16.astype(np.float32).sum(axis=-1)  # (B, P)
    sy = y16.astype(np.float32).sum(axis=-1)
    u = sx @ Wf.T  # (B, P)
    w = sy @ Wa.T + float(L) * ba[None, :]
    urow = u.astype(np.float16)  # (B, P)
    wrow = w.astype(np.float16)

    in_maps = []
    for c in range(NCORES):
        sl = slice(c * BPC, (c + 1) * BPC)
        in_maps.append(
            {
                "x": np.ascontiguousarray(x16[sl]),
                "y": np.ascontiguousarray(y16[sl]),
                "wf": Wf,
                "bf": bf,
                "wa": Wa,
                "ba": ba,
                "urow": np.ascontiguousarray(urow[sl].reshape(1, BPC * P)),
                "wrow": np.ascontiguousarray(wrow[sl].reshape(1, BPC * P)),
            }
        )
    return in_maps


def kernel(x, y, Wf, bf, Wa, ba):
    from concourse.bass_utils import run_bass_kernel_spmd

    in_maps = make_in_maps(x, y, Wf, bf, Wa, ba)
    nc = _get_nc()
    res = run_bass_kernel_spmd(nc, in_maps, core_ids=list(range(NCORES)))
    out = np.concatenate([r["out"] for r in res.results], axis=0)
    return np.ascontiguousarray(out.astype(np.float32))


if __name__ == "__main__":
    rng = np.random.default_rng(0)
    inputs = {
        "x": rng.standard_normal((B, P, L), dtype=np.float32),
        "y": rng.standard_normal((B, P, L), dtype=np.float32),
        "Wf": (rng.standard_normal((P, P)) / np.sqrt(P)).astype(np.float32),
        "bf": (rng.standard_normal(P) * 0.02).astype(np.float32),
        "Wa": (rng.standard_normal((P, P)) / np.sqrt(P)).astype(np.float32),
        "ba": (rng.standard_normal(P) * 0.02).astype(np.float32),
    }
    o = kernel(**inputs)
    print(o.shape, o.dtype)


# revision 75
# speedup vs baseline: 1.0378x; 1.0378x over previous
"""Biaffine attention kernel for Trainium2, data-parallel over 8 NeuronCores.

Math (per batch b, X = x[b] (128, L), Y = y[b]):
    xp = Wf X + bf 1^T,  yp = Wa Y + ba 1^T
    scores = xp yp^T
           = Wf (X Y^T) Wa^T + (Wf X 1) ba^T + bf (Wa Y 1)^T + L bf ba^T
    attn = softmax(scores, -1) / sqrt(L)
    out  = attn (xp + yp) = (attn Wf) X + (attn Wa) Y + (attn (bf+ba)) 1^T

Distribution: batch dim (32) sharded 4-per-core across 8 cores; weights
replicated. No collectives.

Implementation notes (per core):
  - x/y are cast to fp16 on the host; all HBM traffic is fp16 (in 4 MiB +
    out 2 MiB per batch vs 12 MiB for fp32), out is upcast host-side.
  - "gen": xpT chunks = matmul(lhsT=X_c, rhs=WfT) produce the transposed
    projections directly from the natural-layout input — this fuses the
    projection and the transpose into one PE pass (128 N=128 matmuls per
    batch instead of 64 proj-chunk + 128 transpose ops). PSUM evacuation
    is split between ScalarE (x) and VectorE (y).
  - scores accumulate over the 64 transposed chunks into one PSUM bank;
    the three bias rank-1 terms are added exactly via two k=1 matmuls
    (u ba^T and bf (Wa sy + L ba)^T) from host-precomputed rows.
  - out = AfT.T @ X + AaT.T @ Y consumed from the still-resident raw fp16
    inputs (AfT = (attn Wf)^T etc., two small matmuls per batch), with the
    attn(bf+ba) column folded in as the ScalarE PSUM-evacuation bias.
  - software pipelining: gen(b+1) issues between softmax(b) and the PE
    tail of batch b; the second-to-last tail is deferred past the last
    softmax; small constants ride the SWDGE ring so nothing queues behind
    the input stream.
"""

import numpy as np

P = 128
L = 8192
B = 32
NCORES = 8
BPC = B // NCORES  # batches per core
SQRT_L = float(np.sqrt(float(L)))

GEN_GRP = 4  # gen chunks per PSUM evacuation (4*128 fp32 = 1 bank)
OUT_TILE = 2048  # out staging tile (fp16) per DMA store
OUT_CHUNK = 512  # out matmul free dim / PSUM bank
IN_PIECE = 2048  # input DMA piece (cols)


def _patch_tail_drain(tile, mybir, ScopedClock):
    """This container's walrus rejects >1 sync wait on the kernel-tail Drain
    (setupSyncWait: 'Too many sync wait commands'). Spread the tail-drain
    waits across a chain of drains, one wait each."""
    if getattr(tile.TileContext, "_drain_split_patched", False):
        return

    def _split_drain_and_barrier(self, tick_clock, wait_clock):
        nc = self.nc
        drain_inst = nc.sync.drain()
        wait_clock.add_sem_waits(
            drain_inst.ins, ScopedClock({None: tick_clock.global_clock})
        )
        si = drain_inst.ins.sync_info
        if si is not None and si.on_wait is not None and len(si.on_wait) > 1:
            waits = list(si.on_wait)
            si.on_wait = waits[:1]
            # spread the remaining waits across all engines as parallel
            # single-wait NOP chains (walrus allows only one wait per
            # instruction; a serial chain on SP costs ~4us of kernel tail)
            engines = [nc.tensor, nc.vector, nc.scalar, nc.gpsimd, nc.sync]
            for i, w in enumerate(waits[1:]):
                extra = engines[i % len(engines)].nop(nofuse=True)
                esi = extra.ins.sync_info
                if esi is None:
                    extra.ins.sync_info = mybir.SyncInfo(on_wait=[w], on_update=[])
                else:
                    ow = list(esi.on_wait) if esi.on_wait else []
                    ow.append(w)
                    esi.on_wait = ow
        nc.all_engine_barrier()
        assert self.sems is not None
        popped = nc._tile_sem_poison_stack.pop()
        assert popped is self._sem_poison
        nc.clear_and_free_semaphores(list(self.sems.allocated().values()))
        nc.all_engine_barrier()

    tile.TileContext._drain_and_barrier = _split_drain_and_barrier
    tile.TileContext._drain_split_patched = True


def _split_excess_waits(nc, mybir, max_waits=1):
    """Walrus in this container rejects instructions carrying more than a
    couple of sync waits ('Too many sync wait commands'). Hoist excess waits
    onto dedicated same-engine NoOps inserted just before the instruction."""
    ctr = 0
    for blk in nc.m.functions[0].blocks:
        new_insts = []
        for inst in blk.instructions:
            si = inst.sync_info
            if si is not None and si.on_wait and len(si.on_wait) > max_waits:
                waits = list(si.on_wait)
                excess, keep = waits[:-max_waits], waits[-max_waits:]
                si.on_wait = keep
                for i in range(0, len(excess), max_waits):
                    ctr += 1
                    nop = mybir.InstNoOp(
                        name=f"I-waitsplit-{ctr}",
                        sync_info=mybir.SyncInfo(
                            on_wait=excess[i : i + max_waits], on_update=[]
                        ),
                        bass_nofuse=True,
                        engine=inst.engine,
                    )
                    nc.register_instruction(nop)
                    new_insts.append(nop)
            new_insts.append(inst)
        blk.instructions = new_insts


def build_nc(bpc=BPC, seq=L):
    import concourse.bass as bass
    import concourse.mybir as mybir
    import concourse.tile as tile
    from concourse.masks import make_identity
    from concourse.vector_clock import ScopedClock

    _patch_tail_drain(tile, mybir, ScopedClock)

    f32 = mybir.dt.float32
    f16 = mybir.dt.float16
    AF = mybir.ActivationFunctionType
    ALU = mybir.AluOpType
    AX = mybir.AxisListType

    sqrt_l = float(np.sqrt(float(seq)))
    ntr = seq // P  # number of 128-col chunks
    ngrp = ntr // GEN_GRP
    nout = seq // OUT_TILE
    cpo = OUT_TILE // OUT_CHUNK
    npc = seq // IN_PIECE  # input dma pieces

    nc = bass.Bass("TRN2", target_bir_lowering=False, debug=False)
    x_d = nc.dram_tensor("x", [bpc, P, seq], f16, kind="ExternalInput").ap()
    y_d = nc.dram_tensor("y", [bpc, P, seq], f16, kind="ExternalInput").ap()
    wf_d = nc.dram_tensor("wf", [P, P], f32, kind="ExternalInput").ap()
    bf_d = nc.dram_tensor("bf", [P], f32, kind="ExternalInput").ap()
    wa_d = nc.dram_tensor("wa", [P, P], f32, kind="ExternalInput").ap()
    ba_d = nc.dram_tensor("ba", [P], f32, kind="ExternalInput").ap()
    # host-precomputed bias rank-1 rows: u = Wf (X 1), w = Wa (Y 1) + L ba
    urow_d = nc.dram_tensor("urow", [1, bpc * P], f16, kind="ExternalInput").ap()
    wrow_d = nc.dram_tensor("wrow", [1, bpc * P], f16, kind="ExternalInput").ap()
    out_d = nc.dram_tensor("out", [bpc, P, seq], f16, kind="ExternalOutput").ap()

    def load_inputs(b, x_t, y_t):
        if b == 0:
            # graduated pieces: compute can start after ~256 KiB
            edges = sorted({min(e, seq) for e in (0, 512, 1024, 2048, 4096, seq)})
        else:
            edges = [p_ * IN_PIECE for p_ in range(npc + 1)]
        for e0, e1 in zip(edges[:-1], edges[1:]):
            ps_ = slice(e0, e1)
            nc.sync.dma_start(x_t[:, ps_], x_d[b, :, ps_])
            if b == 0:
                # parallel ring for the critical first batch
                nc.scalar.dma_start(y_t[:, ps_], y_d[b, :, ps_])
            else:
                nc.sync.dma_start(y_t[:, ps_], y_d[b, :, ps_])

    with tile.TileContext(nc) as tc:
        with (
            tc.tile_pool(name="consts", bufs=1) as consts,
            tc.tile_pool(name="xin", bufs=3) as xin_pool,
            tc.tile_pool(name="acts", bufs=2) as acts_pool,
            tc.tile_pool(name="sm", bufs=2) as sm_pool,
            tc.tile_pool(name="outs", bufs=3) as out_pool,
            tc.tile_pool(name="pgen", bufs=3, space="PSUM") as psum_gen,
            tc.tile_pool(name="psc", bufs=2, space="PSUM") as psum_sc,
            tc.tile_pool(name="pout", bufs=3, space="PSUM") as psum_out,
        ):
            # ---- critical constants first (ACT ring), then the batch-0
            # preload pieces, then the non-critical constants ----
            wf_nat = consts.tile([P, P], f32)
            nc.scalar.dma_start(wf_nat, wf_d)
            wa_nat = consts.tile([P, P], f32)
            nc.scalar.dma_start(wa_nat, wa_d)
            id32 = consts.tile([P, P], f32)
            make_identity(nc, id32)
            ids16 = consts.tile([P, P], f16)
            make_identity(nc, ids16)

            xy_tiles = {}

            def issue_load(b):
                if b >= bpc:
                    return
                x_t = xin_pool.tile([P, seq], f16, tag="x_t", name="x_t")
                y_t = xin_pool.tile([P, seq], f16, tag="y_t", name="y_t")
                load_inputs(b, x_t, y_t)
                xy_tiles[b] = (x_t, y_t)

            # small constants on the (otherwise idle until the first store)
            # SWDGE ring so they land immediately and never queue behind the
            # batch-0 input pieces
            bias_f = consts.tile([P, 1], f32)
            nc.gpsimd.dma_start(bias_f, bf_d.rearrange("(p o) -> p o", o=1))
            bias_a = consts.tile([P, 1], f32)
            nc.gpsimd.dma_start(bias_a, ba_d.rearrange("(p o) -> p o", o=1))
            bf_row = consts.tile([1, P], f32)
            nc.gpsimd.dma_start(bf_row, bf_d.rearrange("(o p) -> o p", o=1))
            ba_row = consts.tile([1, P], f32)
            nc.gpsimd.dma_start(ba_row, ba_d.rearrange("(o p) -> o p", o=1))
            urows_t = consts.tile([1, bpc * P], f16)
            nc.gpsimd.dma_start(urows_t, urow_d)
            wrows_t = consts.tile([1, bpc * P], f16)
            nc.gpsimd.dma_start(wrows_t, wrow_d)

            issue_load(0)

            for b0 in range(1, min(3, bpc)):
                issue_load(b0)

            # fp16 natural weights (lhsT for the AfT/AaT matmuls)
            wf16 = consts.tile([P, P], f16)
            nc.vector.tensor_copy(out=wf16, in_=wf_nat)
            wa16 = consts.tile([P, P], f16)
            nc.vector.tensor_copy(out=wa16, in_=wa_nat)
            # fp16 transposed weights ([in, out], rhs of the gen matmuls)
            wfT16 = consts.tile([P, P], f16)
            waT16 = consts.tile([P, P], f16)
            for nat, tsp in ((wf_nat, wfT16), (wa_nat, waT16)):
                pwb = psum_sc.tile([P, 4 * P], f32, tag="ps", name="pwb")
                pw = pwb[:, 0:P]
                nc.tensor.transpose(pw, nat, id32)
                nc.vector.tensor_copy(out=tsp, in_=pw)
            # fp16 bias rows, c = bf+ba column (fp16 rhs; fp32 for reference)
            bf_row16 = consts.tile([1, P], f16)
            nc.vector.tensor_copy(out=bf_row16, in_=bf_row)
            ba_row16 = consts.tile([1, P], f16)
            nc.vector.tensor_copy(out=ba_row16, in_=ba_row)
            c_col16 = consts.tile([P, 1], f16)
            nc.vector.scalar_tensor_tensor(
                out=c_col16, in0=bias_f, scalar=1.0, in1=bias_a,
                op0=ALU.mult, op1=ALU.add,
            )

            def do_gen(b):
                # transposed projections (no bias):
                # xpT[:, c*128:(c+1)*128] = X_c^T @ Wf^T   (l on partitions)
                x_t, y_t = xy_tiles[b]
                xpT = acts_pool.tile([P, seq], f16, tag="xpT", name="xpT")
                ypT = acts_pool.tile([P, seq], f16, tag="ypT", name="ypT")
                for g in range(ngrp):
                    for src, wT, dst, eng in (
                        (x_t, wfT16, xpT, nc.scalar),
                        (y_t, waT16, ypT, nc.vector),
                    ):
                        pg = psum_gen.tile([P, GEN_GRP * P], f32, tag="pg", name="pg")
                        for t in range(GEN_GRP):
                            c = g * GEN_GRP + t
                            cs = slice(c * P, (c + 1) * P)
                            ts_ = slice(t * P, (t + 1) * P)
                            nc.tensor.matmul(
                                pg[:, ts_], src[:, cs], wT,
                                start=True, stop=True,
                            )
                        gs = slice(g * GEN_GRP * P, (g + 1) * GEN_GRP * P)
                        if eng is nc.scalar:
                            nc.scalar.activation(
                                out=dst[:, gs], in_=pg, func=AF.Identity
                            )
                        else:
                            nc.vector.tensor_copy(out=dst[:, gs], in_=pg)
                return xpT, ypT

            def do_scores(b, xpT, ypT):
                # scores = sum_c xpT_c^T ypT_c + rank-1 bias terms.
                # One PSUM bank per batch: cols 0-127 scores, the rest is
                # sliced by do_tail for the small post-softmax matmuls.
                pbank = psum_sc.tile([P, 4 * P], f32, tag="ps", name="pbank")
                ps = pbank[:, 0:P]
                for c in range(ntr):
                    cs = slice(c * P, (c + 1) * P)
                    nc.tensor.matmul(
                        ps, xpT[:, cs], ypT[:, cs],
                        start=(c == 0), stop=False,
                    )
                bs = slice(b * P, (b + 1) * P)
                nc.tensor.matmul(
                    ps, urows_t[:, bs], ba_row16, start=False, stop=False
                )
                nc.tensor.matmul(
                    ps, bf_row16, wrows_t[:, bs], start=False, stop=True
                )
                return pbank

            def do_softmax_a(b, pbank):
                # softmax part 1: max-reduce + exp (DVE + ACT)
                ps = pbank[:, 0:P]
                negmx = sm_pool.tile([P, 1], f32, tag="negmx", name="negmx")
                nc.vector.tensor_reduce(
                    out=negmx, in_=ps, axis=AX.X, op=ALU.max, negate=True
                )
                e = sm_pool.tile([P, P], f32, tag="e", name="e")
                se = sm_pool.tile([P, 1], f32, tag="se", name="se")
                nc.scalar.activation(
                    out=e, in_=ps, func=AF.Exp, bias=negmx, scale=1.0, accum_out=se
                )
                return e, se

            def do_softmax_b(b, e, se):
                # softmax part 2: normalize (DVE only)
                sse = sm_pool.tile([P, 1], f32, tag="sse", name="sse")
                nc.vector.tensor_scalar_mul(sse, se, sqrt_l)
                rcp = sm_pool.tile([P, 1], f32, tag="rcp", name="rcp")
                nc.vector.reciprocal(rcp, sse)
                attn = sm_pool.tile([P, P], f16, tag="attn", name="attn")
                nc.vector.tensor_scalar_mul(attn, e, rcp)
                return attn

            def do_softmax(b, pbank):
                e, se = do_softmax_a(b, pbank)
                return do_softmax_b(b, e, se)

            def do_tail(b, attn, pbank, out_tile=OUT_TILE):
                x_t, y_t = xy_tiles.pop(b)
                nout_ = seq // out_tile
                cpo_ = out_tile // OUT_CHUNK
                pat = pbank[:, P : P + P // 2].bitcast(f16)
                nc.tensor.transpose(pat, attn, ids16)
                attnT = sm_pool.tile([P, P], f16, tag="attnT", name="attnT")
                nc.vector.tensor_copy(out=attnT, in_=pat)

                # ---- phase D: folded output weights ----
                # AfT = Wf^T attn^T = (attn Wf)^T ; attnc = attn (bf+ba)
                paf = pbank[:, 2 * P - P // 2 : 3 * P - P // 2]
                nc.tensor.matmul(paf, wf16, attnT, start=True, stop=True)
                afT16 = sm_pool.tile([P, P], f16, tag="afT", name="afT")
                nc.vector.tensor_copy(out=afT16, in_=paf)
                paa = pbank[:, 3 * P - P // 2 : 4 * P - P // 2]
                nc.tensor.matmul(paa, wa16, attnT, start=True, stop=True)
                aaT16 = sm_pool.tile([P, P], f16, tag="aaT", name="aaT")
                nc.vector.tensor_copy(out=aaT16, in_=paa)
                pac = pbank[:, 4 * P - P // 2 : 4 * P - P // 2 + 1]
                nc.tensor.matmul(pac, attnT, c_col16, start=True, stop=True)
                attnc = sm_pool.tile([P, 1], f32, tag="attnc", name="attnc")
                nc.vector.tensor_copy(out=attnc, in_=pac)

                # ---- phase E: out = AfT.T @ X + AaT.T @ Y + attnc ----
                otag = "ot" if out_tile == OUT_TILE else f"ot{out_tile}"
                for h in range(nout_):
                    ot = out_pool.tile([P, out_tile], f16, tag=otag, name="ot")
                    for cc in range(cpo_):
                        c0 = h * out_tile + cc * OUT_CHUNK
                        cs = slice(c0, c0 + OUT_CHUNK)
                        po = psum_out.tile([P, OUT_CHUNK], f32, tag="po", name="po")
                        nc.tensor.matmul(
                            po, afT16, x_t[:, cs], start=True, stop=False
                        )
                        nc.tensor.matmul(
                            po, aaT16, y_t[:, cs], start=False, stop=True
                        )
                        ots = ot[:, cc * OUT_CHUNK : (cc + 1) * OUT_CHUNK]
                        # split PSUM evacuation between ACT and DVE
                        if cc % 2 == 0:
                            nc.scalar.activation(
                                out=ots, in_=po, func=AF.Identity, bias=attnc
                            )
                        else:
                            nc.vector.tensor_scalar_add(ots, po, attnc)
                    hs = slice(h * out_tile, (h + 1) * out_tile)
                    if b == bpc - 1:
                        # last batch: ACT HWDGE ring (idle by then) — keeps
                        # the slow SWDGE drain off the kernel's critical tail
                        nc.scalar.dma_start(out_d[b, :, hs], ot)
                    else:
                        # stores issue from the gpsimd SWDGE ring: ACT is
                        # busy with evacuations, the SP ring with loads
                        nc.gpsimd.dma_start(out_d[b, :, hs], ot)

            # ---- software-pipelined driver ----
            # gen(b+1) is issued between softmax(b) and the PE tail of b,
            # so the PE stays busy during the serial softmax chain. The
            # second-to-last tail is deferred past the last softmax so that
            # one is hidden too.
            xpT, ypT = do_gen(0)
            stash = None
            for b in range(bpc):
                pbank = do_scores(b, xpT, ypT)
                if b + 1 < bpc:
                    attn = do_softmax(b, pbank)
                    issue_load(b + 3)
                    xpT, ypT = do_gen(b + 1)
                    if b == bpc - 2:
                        stash = (attn, pbank)
                    else:
                        do_tail(b, attn, pbank)
                else:
                    # last batch: exp first, then the deferred tail runs on
                    # the PE while the normalize half completes
                    e, se = do_softmax_a(b, pbank)
                    if stash is not None:
                        do_tail(b - 1, stash[0], stash[1])
                    attn = do_softmax_b(b, e, se)
                    do_tail(b, attn, pbank, out_tile=1024)

    _split_excess_waits(nc, mybir, max_waits=1)
    return nc


_nc_cache = {}


def _get_nc():
    key = (BPC, L)
    if key not in _nc_cache:
        _nc_cache[key] = build_nc(BPC, L)
    return _nc_cache[key]


def make_in_maps(x, y, Wf, bf, Wa, ba):
    x16 = np.asarray(x).astype(np.float16)
    y16 = np.asarray(y).astype(np.float16)
    Wf = np.ascontiguousarray(np.asarray(Wf, dtype=np.float32))
    bf = np.ascontiguousarray(np.asarray(bf, dtype=np.float32))
    Wa = np.ascontiguousarray(np.asarray(Wa, dtype=np.float32))
    ba = np.ascontiguousarray(np.asarray(ba, dtype=np.float32))

    # Bias rank-1 rows for the scores (exact, vs the fp16-quantized inputs):
    # scores = Wf G Wa^T + u ba^T + bf w^T with u = Wf (X 1), w = Wa (Y 1) + L ba
    sx = x16.astype(np.float32).sum(axis=-1)  # (B, P)
    sy = y16.astype(np.float32).sum(axis=-1)
    u = sx @ Wf.T  # (B, P)
    w = sy @ Wa.T + float(L) * ba[None, :]
    urow = u.astype(np.float16)  # (B, P)
    wrow = w.astype(np.float16)

    in_maps = []
    for c in range(NCORES):
        sl = slice(c * BPC, (c + 1) * BPC)
        in_maps.append(
            {
                "x": np.ascontiguousarray(x16[sl]),
                "y": np.ascontiguousarray(y16[sl]),
                "wf": Wf,
                "bf": bf,
                "wa": Wa,
                "ba": ba,
                "urow": np.ascontiguousarray(urow[sl].reshape(1, BPC * P)),
                "wrow": np.ascontiguousarray(wrow[sl].reshape(1, BPC * P)),
            }
        )
    return in_maps


def kernel(x, y, Wf, bf, Wa, ba):
    from concourse.bass_utils import run_bass_kernel_spmd

    in_maps = make_in_maps(x, y, Wf, bf, Wa, ba)
    nc = _get_nc()
    res = run_bass_kernel_spmd(nc, in_maps, core_ids=list(range(NCORES)))
    out = np.concatenate([r["out"] for r in res.results], axis=0)
    return np.ascontiguousarray(out.astype(np.float32))


if __name__ == "__main__":
    rng = np.random.default_rng(0)
    inputs = {
        "x": rng.standard_normal((B, P, L), dtype=np.float32),
        "y": rng.standard_normal((B, P, L), dtype=np.float32),
        "Wf": (rng.standard_normal((P, P)) / np.sqrt(P)).astype(np.float32),
        "bf": (rng.standard_normal(P) * 0.02).astype(np.float32),
        "Wa": (rng.standard_normal((P, P)) / np.sqrt(P)).astype(np.float32),
        "ba": (rng.standard_normal(P) * 0.02).astype(np.float32),
    }
    o = kernel(**inputs)
    print(o.shape, o.dtype)
